# revision 21
# baseline (speedup 1.0000x reference)
"""MultiHeadAttention Trainium2 kernel, 8-way tensor-parallel over heads.

B=4, T=2048, C=1024, H=16 heads, Dh=64. Each of the 8 NeuronCores owns 2
heads. Per batch: QKV projection (bf16 matmuls, fp32 PSUM accumulation),
attention in the transposed layout (S^T = K_tile^T @ Q^T contracted over Dh;
exp on ScalarE writing fp8e4m3; P@V as fp8 DoubleRow matmuls contracting
pairs of 128-key tiles, with an appended ones column yielding the softmax
denominator), and a partial out-projection against this core's 128 columns
of W_out. Host sums the 8 bf16 partials in fp64 and adds b_out.

The attention kt loop is the pacing loop (ScalarE exp ~1.25us/kt). PV for
query-tile qt runs one qt behind S/exp, and QKV for batch b+1 plus the
out-projection for batch b-1 are interleaved into the kt loop as "filler"
PE work so the TensorE never waits on ScalarE.
"""
import sys
sys.path.insert(0, '/opt/trn_rl_repo')
import numpy as np
import ml_dtypes

import concourse.bass as bass
import concourse.mybir as mybir
import concourse.tile as tile
from concourse import bacc
from concourse.bass_utils import run_bass_kernel_spmd
from concourse.masks import make_identity

F32 = mybir.dt.float32
BF16 = mybir.dt.bfloat16
FP8 = mybir.dt.float8e4
AF = mybir.ActivationFunctionType
DR = mybir.MatmulPerfMode.DoubleRow

B, T, C = 4, 2048, 1024
H, DH = 16, 64
NCORES = 8
HPC = H // NCORES          # heads per core (2)
D2 = HPC * DH              # 128, local concat dim
BT = B * T                 # 8192
NT = T // 512              # q/t tiles of 512 per batch (4)
NK = T // 128              # k tiles of 128 per batch (16)
NP = NK // 2               # k-tile pairs (8)
CCH = C // 128             # contraction chunks (8)

PV_FP8 = False              # fp8e4m3 E/V + DoubleRow PV matmuls
FILLERS = True             # interleave QKV(b+1)/outproj(b-1) into the kt loop
BARRIER = False             # debug: serialize qt iterations with engine barriers

E_DT = FP8 if PV_FP8 else BF16
V_DT = FP8 if PV_FP8 else BF16

_NC_CACHE = {}


def build_nc():
    nc = bacc.Bacc()

    xp = nc.dram_tensor("xp", [128, B * NT, CCH, 512], BF16, kind="ExternalInput")
    wq = nc.dram_tensor("wq", [128, CCH, D2], BF16, kind="ExternalInput")
    wk = nc.dram_tensor("wk", [128, CCH, D2], BF16, kind="ExternalInput")
    wv = nc.dram_tensor("wv", [128, CCH, D2], BF16, kind="ExternalInput")
    bq = nc.dram_tensor("bq", [D2, 1], F32, kind="ExternalInput")
    bk = nc.dram_tensor("bk", [D2, 1], F32, kind="ExternalInput")
    bv = nc.dram_tensor("bv", [D2, 1], F32, kind="ExternalInput")
    wo = nc.dram_tensor("wo", [128, C], BF16, kind="ExternalInput")
    y = nc.dram_tensor("y", [BT, C], BF16, kind="ExternalOutput")

    with tile.TileContext(nc) as tc:
        with (
            tc.tile_pool(name="singles", bufs=1) as singles,
            tc.tile_pool(name="xin", bufs=8) as xin,
            tc.tile_pool(name="qkv", bufs=2) as qkv,
            tc.tile_pool(name="vtmp", bufs=2) as vtmp_pool,
            tc.tile_pool(name="esb", bufs=18 if PV_FP8 else 16) as esb,
            tc.tile_pool(name="rsb", bufs=2) as rsb,
            tc.tile_pool(name="osb", bufs=2) as osb,
            tc.tile_pool(name="outsb", bufs=3) as outsb,
            # 8 PSUM banks: s2 2x2 + pv0/pv1 1x1 each + shared 2x1
            tc.tile_pool(name="s2_ps", bufs=2, space="PSUM") as s2_ps,
            tc.tile_pool(name="sh_ps", bufs=2, space="PSUM") as sh_ps,
            tc.tile_pool(name="pv_ps", bufs=1, space="PSUM") as pv_ps,
        ):
            ident = singles.tile([128, 128], BF16)
            make_identity(nc, ident)
            warm_f = singles.tile([128, 512], F32, tag="warm_f")
            nc.vector.memset(warm_f, 1.0)
            warm_b = singles.tile([128, 512], BF16, tag="warm_b")
            nc.vector.tensor_copy(out=warm_b, in_=warm_f)
            for wi in range(12):
                wps = sh_ps.tile([128, 512], F32, tag="sm", name=f"warm{wi}")
                nc.tensor.matmul(out=wps, lhsT=warm_b[:, 0:128], rhs=warm_b,
                                 start=True, stop=True)
            ones16 = singles.tile([128, NK, 1], F32)
            nc.vector.memset(ones16, 1.0)

            wq_sb = singles.tile([128, CCH, D2], BF16, tag="wq")
            wk_sb = singles.tile([128, CCH, D2], BF16, tag="wk")
            wv_sb = singles.tile([128, CCH, D2], BF16, tag="wv")
            for w_dram, w_sb in ((wq, wq_sb), (wk, wk_sb), (wv, wv_sb)):
                nc.sync.dma_start(out=w_sb, in_=w_dram[:, :, :])
            bq_sb = singles.tile([D2, 1], F32, tag="bq")
            bk_sb = singles.tile([D2, 1], F32, tag="bk")
            bv_sb = singles.tile([D2, 1], F32, tag="bv")
            nc.sync.dma_start(out=bq_sb, in_=bq[:, :])
            nc.sync.dma_start(out=bk_sb, in_=bk[:, :])
            nc.sync.dma_start(out=bv_sb, in_=bv[:, :])
            wo_sb = singles.tile([128, C], BF16, tag="wo")
            nc.sync.dma_start(out=wo_sb, in_=wo[:, :])

            xt_tiles = {}

            def ensure_x(b, tt):
                if (b, tt) not in xt_tiles:
                    xt = xin.tile([128, CCH, 512], BF16, tag="xt")
                    nc.sync.dma_start(out=xt, in_=xp[:, b * NT + tt, :, :])
                    xt_tiles[(b, tt)] = xt
                return xt_tiles[(b, tt)]

            qkv_tiles = {}

            def qkv_steps(b):
                """QKV projection for batch b; yields between PE chunks."""
                qT = qkv.tile([D2, T], BF16, tag="q")
                kT = qkv.tile([D2, T], BF16, tag="k")
                # per k-tile lhsT layout (208 cols; 208 = 16*13 keeps the
                # DoubleRow pair-dim stride 16B-aligned for dual-fp8 ldweights):
                #   h0: cols 0:128  = [V_h0 | 1 | junk63]  (M=128: num@0:64, Z@64,
                #        junk rows 65:128 never read; full-width lhsT avoids the
                #        slow 65-col ldweights)
                #   h1: cols 80:208 = [junk32 | 1@112 | junk31 | V_h1@144] (M=128:
                #        Z@32, num@64:128; junk cols make junk PSUM rows, never read)
                v1 = qkv.tile([128, NK, 208], V_DT, tag="v")
                qkv_tiles[b] = (qT, kT, v1)
                nc.gpsimd.memset(v1, 0.0)
                nc.vector.tensor_copy(out=v1[:, :, DH:DH + 1], in_=ones16)
                nc.vector.tensor_copy(out=v1[:, :, 112:113], in_=ones16)
                for tt in range(NT):
                    t0 = tt * 512
                    xt = ensure_x(b, tt)
                    for w_sb, b_sb, dest in ((wk_sb, bk_sb, kT), (wq_sb, bq_sb, qT)):
                        ps = sh_ps.tile([128, 512], F32, tag="sm")
                        for ci in range(CCH):
                            nc.tensor.matmul(out=ps, lhsT=w_sb[:, ci, :],
                                             rhs=xt[:, ci, :],
                                             start=(ci == 0), stop=(ci == CCH - 1))
                            if ci % 2 == 1:
                                yield
                        nc.vector.tensor_scalar_add(out=dest[:, t0:t0 + 512],
                                                    in0=ps, scalar1=b_sb)
                        yield
                    ps = sh_ps.tile([128, 512], F32, tag="sm")
                    for ci in range(CCH):
                        nc.tensor.matmul(out=ps, lhsT=wv_sb[:, ci, :],
                                         rhs=xt[:, ci, :],
                                         start=(ci == 0), stop=(ci == CCH - 1))
                        if ci % 2 == 1:
                            yield
                    vt = vtmp_pool.tile([128, 512], BF16)
                    nc.vector.tensor_scalar_add(out=vt, in0=ps, scalar1=bv_sb)
                    yield
                    for s in range(4):
                        tp = sh_ps.tile([128, 512], BF16, tag="sm")
                        nc.tensor.transpose(out=tp[:, 0:128],
                                            in_=vt[:, s * 128:(s + 1) * 128],
                                            identity=ident)
                        kt = tt * 4 + s
                        sl = v1[:, kt, :]
                        dst = bass.AP(tensor=sl.tensor, offset=sl.offset,
                                      ap=[list(sl.ap[0]), [144, 2], [1, DH]])
                        nc.vector.tensor_copy(
                            out=dst,
                            in_=tp[:, 0:128].rearrange("p (g x) -> p g x", g=2))
                        yield

            oT2_tiles = {}

            def outproj_steps(b):
                """Partial out-projection for batch b; yields between PE ops."""
                oT2 = oT2_tiles.pop(b)
                for ts in range(T // 128):
                    ot = outsb.tile([128, C], BF16)
                    for n in range(2):
                        n0 = n * 512
                        ops = sh_ps.tile([128, 512], F32, tag="sm")
                        nc.tensor.matmul(
                            out=ops,
                            lhsT=oT2[:, ts * 128:(ts + 1) * 128],
                            rhs=wo_sb[:, n0:n0 + 512],
                            start=True, stop=True)
                        nc.vector.tensor_copy(out=ot[:, n0:n0 + 512], in_=ops)
                        yield
                    nc.sync.dma_start(
                        out=y[b * T + ts * 128:b * T + (ts + 1) * 128, :],
                        in_=ot)

            def evac_normalize(oT2, q0, pv0, pv1):
                # h0: num@pv0[0:64], Z@pv0[64]; h1: num@pv1[64:128], Z@pv1[32]
                pvc = rsb.tile([128, 512], BF16, tag="pvc")
                nc.vector.tensor_copy(out=pvc[0:DH, :], in_=pv0[0:DH, :])
                nc.vector.tensor_copy(out=pvc[DH:128, :], in_=pv1[DH:128, :])
                z0 = rsb.tile([1, 512], F32, tag="z0", bufs=1)
                nc.vector.tensor_copy(out=z0, in_=pv0[DH:DH + 1, :])
                z1 = rsb.tile([1, 512], F32, tag="z1", bufs=1)
                nc.vector.tensor_copy(out=z1, in_=pv1[32:33, :])
                r0 = rsb.tile([1, 512], F32, tag="r0", bufs=1)
                nc.vector.reciprocal_approx_fast(out=r0, in_=z0)
                r1 = rsb.tile([1, 512], F32, tag="r1", bufs=1)
                nc.vector.reciprocal_approx_fast(out=r1, in_=z1)
                r0b = rsb.tile([1, 512], BF16, tag="r0b", bufs=1)
                nc.vector.tensor_copy(out=r0b, in_=r0)
                r1b = rsb.tile([1, 512], BF16, tag="r1b", bufs=1)
                nc.vector.tensor_copy(out=r1b, in_=r1)
                rbc = rsb.tile([128, 512], BF16, tag="rbc")
                nc.gpsimd.partition_broadcast(rbc, r1b)
                nc.gpsimd.partition_broadcast(rbc[0:DH, :], r0b)
                nc.vector.tensor_mul(out=oT2[:, q0:q0 + 512], in0=pvc, in1=rbc)

            lh = ((0, 128), (80, 208))

            def pv_pair(v1, ets, pi, pvs):
                for h in range(HPC):
                    if PV_FP8:
                        nc.tensor.matmul(
                            out=pvs[h],
                            lhsT=v1[:, 2 * pi:2 * pi + 2, lh[h][0]:lh[h][1]],
                            rhs=ets[pi][:, :, h * 512:(h + 1) * 512],
                            start=(pi == 0), stop=(pi == NP - 1),
                            perf_mode=DR)
                    else:
                        for j in range(2):
                            kt = 2 * pi + j
                            nc.tensor.matmul(
                                out=pvs[h],
                                lhsT=v1[:, kt, lh[h][0]:lh[h][1]],
                                rhs=ets[pi][:, j, h * 512:(h + 1) * 512],
                                start=(kt == 0), stop=(kt == NK - 1))

            # ---- prologue: prefetch x, project QKV(0) through tt=0 only;
            # the rest streams in as filler under attention(0) with progress
            # guards (the in-order PE queue needs producer instructions
            # emitted before their consumers).
            for tt in range(NT):
                ensure_x(0, tt)
            SPT = 19  # qkv_steps yields per tt: k5 + q5 + v5 + transposes4
            g0 = qkv_steps(0)
            g0_n = [0]

            def g0_pull_until(target):
                while g0_n[0] < target:
                    try:
                        next(g0)
                        g0_n[0] += 1
                    except StopIteration:
                        g0_n[0] = 1 << 30
            g0_pull_until(SPT)

            for b in range(B):
                qT, kT, v1 = qkv_tiles.pop(b)
                oT2 = osb.tile([128, T], BF16, tag="o2")
                oT2_tiles[b] = oT2
                for tt in range(NT):
                    if b + 1 < B:
                        ensure_x(b + 1, tt)

                fillers = []
                tail_fillers = []
                if b == 0:
                    def g0_counted():
                        for _ in g0:
                            g0_n[0] += 1
                            yield
                    fillers.append(g0_counted())
                if b > 0:
                    fillers.append(outproj_steps(b - 1))
                if b + 1 < B:
                    fillers.append(qkv_steps(b + 1))
                else:
                    # last batch: its own out-projection, pulled only in the
                    # tail (its inputs exist once the early evacs are done)
                    tail_fillers.append(outproj_steps(b))

                def pull(n, force=False, lists=(fillers,)):
                    if not FILLERS and not force:
                        return
                    for _ in range(n):
                        cur = None
                        for li in lists:
                            if li:
                                cur = li
                                break
                        if cur is None:
                            return
                        try:
                            next(cur[0])
                        except StopIteration:
                            cur.pop(0)

                prev = None  # (q0, ets, pv0, pv1) of the previous query tile
                for qt in range(NT):
                    q0 = qt * 512
                    ets = [esb.tile([128, 2, 1024], E_DT, tag="et",
                                    name=f"et{b}_{qt}_{pi}")
                           for pi in range(NP)]
                    pv0 = pv_ps.tile([128, 512], F32, tag="pv0")
                    pv1 = pv_ps.tile([128, 512], F32, tag="pv1")
                    for kt in range(NK):
                        if b == 0 and qt == 0:
                            g0_pull_until(SPT * (kt // 4) + 5)
                        if b == 0 and qt <= 1 and prev is not None and kt % 2 == 1:
                            g0_pull_until(SPT * ((kt // 2) // 2) + SPT)
                        s2 = s2_ps.tile([128, 1024], F32, tag="s2")
                        for h in range(HPC):
                            hs = h * DH
                            nc.tensor.matmul(
                                out=s2[:, h * 512:(h + 1) * 512],
                                lhsT=kT[hs:hs + DH, kt * 128:(kt + 1) * 128],
                                rhs=qT[hs:hs + DH, q0:q0 + 512],
                                start=True, stop=True)
                        if prev is not None and kt % 2 == 1:
                            pv_pair(v1, prev[1], kt // 2, (prev[2], prev[3]))
                        nc.scalar.activation(out=ets[kt // 2][:, kt % 2],
                                             in_=s2, func=AF.Exp, scale=0.125)
                        pull(2)
                    if prev is not None:
                        evac_normalize(oT2, prev[0], prev[2], prev[3])
                    prev = (q0, ets, pv0, pv1)
                    if BARRIER:
                        tc.strict_bb_all_engine_barrier()
                # tail: PV + normalize for the last query tile
                for pi in range(NP):
                    pv_pair(v1, prev[1], pi, (prev[2], prev[3]))
                    pull(2, lists=(fillers, tail_fillers))
                evac_normalize(oT2, prev[0], prev[2], prev[3])
                # drain leftover filler work
                pull(1 << 30, force=True, lists=(fillers, tail_fillers))
                if BARRIER:
                    tc.strict_bb_all_engine_barrier()


    nc.compile()
    return nc


def make_in_maps(x, W_qkv, b_qkv, W_out, b_out):
    # x pre-tiled to the exact SBUF layout: xp[p, tile, ci, c] = x[tile*512+c, ci*128+p]
    xp = np.ascontiguousarray(
        x.reshape(B * NT, 512, CCH, 128).transpose(3, 0, 2, 1)
    ).astype(ml_dtypes.bfloat16)
    in_maps = []
    for c in range(NCORES):
        r0 = c * D2
        def wshuf(wslice):
            # [D2, C] weight rows -> lhsT chunks [128 p, CCH, D2]
            return np.ascontiguousarray(
                wslice.T.reshape(CCH, 128, D2).transpose(1, 0, 2)
            ).astype(ml_dtypes.bfloat16)
        wq = wshuf(W_qkv[r0:r0 + D2, :])
        wk = wshuf(W_qkv[C + r0:C + r0 + D2, :])
        wv = wshuf(W_qkv[2 * C + r0:2 * C + r0 + D2, :])
        bqc = np.ascontiguousarray(b_qkv[r0:r0 + D2].reshape(D2, 1))
        bkc = np.ascontiguousarray(b_qkv[C + r0:C + r0 + D2].reshape(D2, 1))
        bvc = np.ascontiguousarray(b_qkv[2 * C + r0:2 * C + r0 + D2].reshape(D2, 1))
        woc = np.ascontiguousarray(W_out[:, r0:r0 + D2].T).astype(ml_dtypes.bfloat16)
        in_maps.append({
            "xp": xp, "wq": wq, "wk": wk, "wv": wv,
            "bq": bqc, "bk": bkc, "bv": bvc, "wo": woc,
        })
    return in_maps


def run(x, W_qkv, b_qkv, W_out, b_out, trace=False):
    if "nc" not in _NC_CACHE:
        _NC_CACHE["nc"] = build_nc()
    nc = _NC_CACHE["nc"]
    in_maps = make_in_maps(
        np.asarray(x, dtype=np.float32), np.asarray(W_qkv, dtype=np.float32),
        np.asarray(b_qkv, dtype=np.float32), np.asarray(W_out, dtype=np.float32),
        np.asarray(b_out, dtype=np.float32))
    res = run_bass_kernel_spmd(nc, in_maps, core_ids=list(range(NCORES)),
                               trace=trace)
    acc = np.zeros((BT, C), dtype=np.float64)
    for c in range(NCORES):
        acc += res.results[c]["y"].astype(np.float64)
    acc += np.asarray(b_out, dtype=np.float64)
    out = acc.astype(np.float32).reshape(B, T, C)
    return out, res


def kernel(x, W_qkv, b_qkv, W_out, b_out):
    out, _ = run(x, W_qkv, b_qkv, W_out, b_out, trace=False)
    return out
